# revision 1
# baseline (speedup 1.0000x reference)
"""AttentiveRNNLanguageModel Trainium2 kernel (8-core SPMD).

Sharding: the sequential LSTMs + positional attention are replicated on all
8 cores (per-step collectives have ~10us floors, so the 1024-step recurrence
cannot be sharded); the tied embedding/decoder matmul [V,H] is sharded
vocab-wise 8 ways (4000 vocab per core). The host concatenates logit shards.
No collectives.

Device layout is "transposed": LSTM state hT/cT live as [128, 16]
(partition = h-dim within a 128-chunk, free = 4*chunk + batch), gates
reordered to (i, f, o, g). Per step, 64 weight-stationary bf16 matmuls
accumulate gT [128, 64] in PSUM; the positional LSTM (per-gate split so all
elementwise work stays partition-aligned) and the mu scan are fused into the
same For_i loop. Post phases: Gaussian attention weights with a
host-precomputed masked rel grid, L1 normalization via ones-matmul column
sums, then ctx^T / combined^T / decoder matmuls in bf16.
"""
import os
import numpy as np
import ml_dtypes
from contextlib import ExitStack

import concourse.bass as bass
import concourse.tile as tile
from concourse import bacc, mybir
from concourse.bass_utils import run_bass_kernel_spmd

F32 = mybir.dt.float32
BF16 = mybir.dt.bfloat16
AF = mybir.ActivationFunctionType

B, T, H, P, V = 4, 1024, 512, 20, 32000
NCORES = 8
VSH = V // NCORES
EPS_SIG = 0.001
EPS_NORM = 1e-12
NBLK, SPB = 64, 16

LAST_EXEC_NS = [None]


def _bf(x):
    return np.ascontiguousarray(np.asarray(x).astype(ml_dtypes.bfloat16))


def _f32(x):
    return np.ascontiguousarray(np.asarray(x), dtype=np.float32)


def build_nc():
    nc = bacc.Bacc()
    dt = nc.dram_tensor
    xT_in = dt("xT", [128, 4 * B * T], BF16, kind="ExternalInput")
    wihT_in = dt("wihT", [128, 4 * 16 * 128], BF16, kind="ExternalInput")
    whhT_in = dt("whhT", [128, 4 * 16 * 128], BF16, kind="ExternalInput")
    mbias_in = dt("mbias", [128, 16], F32, kind="ExternalInput")
    wpihT_in = dt("wpihT", [128, 4 * 4 * P], BF16, kind="ExternalInput")
    wphhT_in = dt("wphhT", [P, 4 * P], BF16, kind="ExternalInput")
    w3T_in = dt("w3T", [P, 4], BF16, kind="ExternalInput")
    bp_in = dt("bp", [P, 16], F32, kind="ExternalInput")
    bm_in = dt("bm", [1, 16], F32, kind="ExternalInput")
    invL_in = dt("invL", [1, 4], F32, kind="ExternalInput")
    j1_in = dt("j1", [1, T], F32, kind="ExternalInput")
    relM_in = dt("relM", [128, 8 * T], F32, kind="ExternalInput")
    wcT_in = dt("wcT", [128, 8 * 4 * 128], BF16, kind="ExternalInput")
    bc_in = dt("bc", [128, 4], F32, kind="ExternalInput")
    embT_in = dt("embT", [128, 4 * VSH], BF16, kind="ExternalInput")
    logits_out = dt("logits", [B * T, VSH], F32, kind="ExternalOutput")
    xwt = dt("xwt", [128, T * 64], F32, kind="Internal")

    with tile.TileContext(nc) as tc, ExitStack() as ctx:
        live = ctx.enter_context(tc.tile_pool(name="live", bufs=1))
        encT = live.tile([128, T * 16], BF16)
        mustack = live.tile([128, 4 * T], F32)
        denstack = live.tile([128, 4 * T], F32)

        # ================= Phase 1: bulk xw^T ===============================
        with ExitStack() as p1:
            p1w = p1.enter_context(tc.tile_pool(name="p1w", bufs=1))
            p1e = p1.enter_context(tc.tile_pool(name="p1e", bufs=4))
            p1ps = p1.enter_context(tc.tile_pool(name="p1ps", bufs=6, space="PSUM"))
            xT_sb = p1w.tile([128, 4 * B * T], BF16)
            nc.sync.dma_start(xT_sb[:], xT_in[:, :])
            wih_sb = p1w.tile([128, 4 * 16 * 128], BF16)
            nc.sync.dma_start(wih_sb[:], wihT_in[:, :])
            mb_sb = p1w.tile([128, 16], F32)
            nc.sync.dma_start(mb_sb[:], mbias_in[:, :])
            for mc in range(16):
                for h2 in range(2):
                    pss = [p1ps.tile([128, 512], F32, tag="p1ps", name=f"pss{i}") for i in range(B)]
                    for k in range(4):
                        for b in range(B):
                            nc.tensor.matmul(
                                pss[b][:],
                                wih_sb[:, (k * 16 + mc) * 128:(k * 16 + mc + 1) * 128],
                                xT_sb[:, 4096 * k + 1024 * b + 512 * h2:
                                      4096 * k + 1024 * b + 512 * h2 + 512],
                                start=(k == 0), stop=(k == 3))
                    for b in range(B):
                        ev = p1e.tile([128, 512], F32)
                        nc.scalar.activation(ev[:], pss[b][:], AF.Identity,
                                             bias=mb_sb[:, mc:mc + 1])
                        cc = 4 * mc + b
                        nc.sync.dma_start(
                            xwt[:, 1024 * cc + 512 * h2:1024 * cc + 512 * h2 + 512],
                            ev[:])

        # ================= Phase 2: fused recurrence ========================
        with ExitStack() as p2:
            p2w = p2.enter_context(tc.tile_pool(name="p2w", bufs=1))
            whh_sb = p2w.tile([128, 4 * 16 * 128], BF16)
            nc.sync.dma_start(whh_sb[:], whhT_in[:, :])
            wpih_sb = p2w.tile([128, 4 * 4 * P], BF16)
            nc.sync.dma_start(wpih_sb[:], wpihT_in[:, :])
            wphh_sb = p2w.tile([128, 4 * P], BF16)
            nc.sync.dma_start(wphh_sb[0:P, :], wphhT_in[:, :])
            w3_sb = p2w.tile([128, 4], BF16)
            nc.sync.dma_start(w3_sb[0:P, :], w3T_in[:, :])
            bp_sb = p2w.tile([128, 16], F32)
            nc.sync.dma_start(bp_sb[0:P, :], bp_in[:, :])
            bm_sb = p2w.tile([128, 16], F32)
            nc.sync.dma_start(bm_sb[0:1, :], bm_in[:, :])
            invL_sb = p2w.tile([128, 4], F32)
            nc.sync.dma_start(invL_sb[0:1, :], invL_in[:, :])
            j1_sb = p2w.tile([128, T], F32)
            nc.sync.dma_start(j1_sb[0:1, :], j1_in[:, :])

            h16 = p2w.tile([128, 16], BF16)
            c_sb = p2w.tile([128, 16], F32)
            hp16 = p2w.tile([128, 4], BF16)
            cp_sb = p2w.tile([128, 4], F32)
            mu_sb = p2w.tile([128, 4], F32)
            nc.vector.memset(h16[:], 0.0)
            nc.vector.memset(c_sb[:], 0.0)
            nc.vector.memset(hp16[0:P, :], 0.0)
            nc.vector.memset(cp_sb[0:P, :], 0.0)
            nc.vector.memset(mu_sb[0:1, :], 0.0)

            xw_pool = p2.enter_context(tc.tile_pool(name="xw", bufs=2))
            work = p2.enter_context(tc.tile_pool(name="work", bufs=2))
            gps_pool = p2.enter_context(tc.tile_pool(name="gps", bufs=2, space="PSUM"))
            pps_pool = p2.enter_context(tc.tile_pool(name="pps", bufs=2, space="PSUM"))
            pms_pool = p2.enter_context(tc.tile_pool(name="pms", bufs=2, space="PSUM"))

            xwt_v = xwt[:, :].rearrange("p (cc t) -> p cc t", cc=64)
            with tc.For_i(0, NBLK) as it:
                xw_tile = xw_pool.tile([128, SPB * 64], F32)
                nc.sync.dma_start(
                    xw_tile[:].rearrange("p (cc t) -> p cc t", cc=64),
                    xwt_v[:, :, bass.ds(it * SPB, SPB)])
                xw_v = xw_tile[:].rearrange("p (cc t) -> p cc t", cc=64)
                for s in range(SPB):
                    g_ps = gps_pool.tile([128, 64], F32)
                    for mc in range(16):
                        for k in range(4):
                            nc.tensor.matmul(
                                g_ps[:, 4 * mc:4 * mc + 4],
                                whh_sb[:, (k * 16 + mc) * 128:(k * 16 + mc + 1) * 128],
                                h16[:, 4 * k:4 * k + 4],
                                start=(k == 0), stop=(k == 3))
                    gsum = work.tile([128, 64], F32)
                    nc.vector.tensor_add(gsum[:], g_ps[:], xw_v[:, :, s])
                    sig = work.tile([128, 48], F32)
                    nc.scalar.activation(sig[:], gsum[:, 0:48], AF.Sigmoid)
                    tg = work.tile([128, 16], F32)
                    nc.scalar.activation(tg[:], gsum[:, 48:64], AF.Tanh)
                    t1 = work.tile([128, 16], F32)
                    nc.vector.tensor_mul(t1[:], sig[:, 16:32], c_sb[:])
                    t2 = work.tile([128, 16], F32)
                    nc.vector.tensor_mul(t2[:], sig[:, 0:16], tg[:])
                    nc.vector.tensor_add(c_sb[:], t1[:], t2[:])
                    tct = work.tile([128, 16], F32)
                    nc.scalar.activation(tct[:], c_sb[:], AF.Tanh)
                    hf = work.tile([128, 16], F32)
                    nc.vector.tensor_mul(hf[:], sig[:, 32:48], tct[:])
                    nc.scalar.copy(h16[:], hf[:])
                    nc.vector.tensor_copy(
                        encT[:, bass.ds(it * (SPB * 16) + s * 16, 16)], hf[:])

                    # positional LSTM, per-gate
                    pps = pps_pool.tile([128, 16], F32)
                    for g in range(4):
                        for k in range(4):
                            nc.tensor.matmul(
                                pps[0:P, 4 * g:4 * g + 4],
                                wpih_sb[:, 80 * k + P * g:80 * k + P * g + P],
                                h16[:, 4 * k:4 * k + 4],
                                start=(k == 0), stop=False)
                        nc.tensor.matmul(
                            pps[0:P, 4 * g:4 * g + 4],
                            wphh_sb[0:P, P * g:P * g + P],
                            hp16[0:P, 0:4],
                            start=False, stop=True)
                    gp = work.tile([128, 16], F32)
                    nc.vector.tensor_add(gp[0:P, :], pps[0:P, :], bp_sb[0:P, :])
                    sp = work.tile([128, 12], F32)
                    nc.scalar.activation(sp[0:P, :], gp[0:P, 0:12], AF.Sigmoid)
                    tp = work.tile([128, 4], F32)
                    nc.scalar.activation(tp[0:P, :], gp[0:P, 12:16], AF.Tanh)
                    u1 = work.tile([128, 4], F32)
                    nc.vector.tensor_mul(u1[0:P, :], sp[0:P, 4:8], cp_sb[0:P, :])
                    u2 = work.tile([128, 4], F32)
                    nc.vector.tensor_mul(u2[0:P, :], sp[0:P, 0:4], tp[0:P, :])
                    nc.vector.tensor_add(cp_sb[0:P, :], u1[0:P, :], u2[0:P, :])
                    tcp = work.tile([128, 4], F32)
                    nc.scalar.activation(tcp[0:P, :], cp_sb[0:P, :], AF.Tanh)
                    hpf = work.tile([128, 4], F32)
                    nc.vector.tensor_mul(hpf[0:P, :], sp[0:P, 8:12], tcp[0:P, :])
                    nc.scalar.copy(hp16[0:P, :], hpf[0:P, :])

                    # mw / sigma / mu / den
                    pms = pms_pool.tile([128, 16], F32)
                    for r in range(4):
                        nc.tensor.matmul(pms[0:1, 4 * r:4 * r + 4],
                                         w3_sb[0:P, r:r + 1], hp16[0:P, 0:4],
                                         start=True, stop=True)
                    ms = work.tile([128, 16], F32)
                    nc.vector.tensor_add(ms[0:1, :], pms[0:1, :], bm_sb[0:1, :])
                    rl = work.tile([128, 12], F32)
                    nc.scalar.activation(rl[0:1, :], ms[0:1, 0:12], AF.Relu)
                    sg = work.tile([128, 4], F32)
                    nc.scalar.activation(sg[0:1, :], ms[0:1, 12:16], AF.Sigmoid)
                    sq = work.tile([128, 4], F32)
                    nc.scalar.activation(sq[0:1, :], sg[0:1, :], AF.Square)
                    nc.vector.tensor_scalar(
                        denstack[0:1, bass.ds(it * (SPB * 4) + 4 * s, 4)],
                        sq[0:1, :], 2.0, EPS_SIG,
                        mybir.AluOpType.mult, mybir.AluOpType.add)
                    v1 = work.tile([128, 4], F32)
                    nc.vector.tensor_scalar_mul(
                        v1[0:1, :], rl[0:1, 8:12],
                        j1_sb[0:1, bass.ds(it * SPB + s, 1)])
                    v2 = work.tile([128, 4], F32)
                    nc.vector.tensor_add(v2[0:1, :], rl[0:1, 4:8], v1[0:1, :])
                    v3 = work.tile([128, 4], F32)
                    nc.vector.tensor_mul(v3[0:1, :], v2[0:1, :], invL_sb[0:1, :])
                    v4 = work.tile([128, 4], F32)
                    nc.vector.tensor_mul(v4[0:1, :], rl[0:1, 0:4], mu_sb[0:1, :])
                    nc.vector.tensor_add(mu_sb[0:1, :], v4[0:1, :], v3[0:1, :])
                    nc.vector.tensor_copy(
                        mustack[0:1, bass.ds(it * (SPB * 4) + 4 * s, 4)],
                        mu_sb[0:1, :])

        encT_v = encT[:, :].rearrange("p (t x) -> p t x", x=16)
        nc.vector.reciprocal(denstack[0:1, :], denstack[0:1, :])
        mu_v = mustack[0:1, :].rearrange("o (t b) -> o t b", b=4)
        den_v = denstack[0:1, :].rearrange("o (t b) -> o t b", b=4)

        ctx_pool = ctx.enter_context(tc.tile_pool(name="ctxp", bufs=1))
        ctxTs = [ctx_pool.tile([128, 4 * T], BF16, tag=f"ctxT{b}", name=f"ctxT{b}") for b in range(B)]

        # ================= Phase 3a: attention ==============================
        with ExitStack() as p3:
            cpool = p3.enter_context(tc.tile_pool(name="p3c", bufs=1))
            relM_sb = cpool.tile([128, 8 * T], F32)
            nc.sync.dma_start(relM_sb[:], relM_in[:, :])
            ident = cpool.tile([128, 128], BF16)
            from concourse.masks import make_identity
            make_identity(nc, ident[:])
            ones_col = cpool.tile([128, 1], BF16)
            nc.vector.memset(ones_col[:], 1.0)
            ones_row = cpool.tile([128, 128], F32)
            nc.vector.memset(ones_row[0:1, :], 1.0)

            bpool = p3.enter_context(tc.tile_pool(name="p3b", bufs=1))
            wk = p3.enter_context(tc.tile_pool(name="p3wk", bufs=2))
            nrm = p3.enter_context(tc.tile_pool(name="p3n", bufs=1))
            tps_pool = p3.enter_context(tc.tile_pool(name="tpsp", bufs=2, space="PSUM"))
            ps512 = p3.enter_context(tc.tile_pool(name="ps512", bufs=2, space="PSUM"))
            rowps = p3.enter_context(tc.tile_pool(name="rowps", bufs=2, space="PSUM"))

            for b in range(B):
                muB = bpool.tile([128, T], F32, tag="muB")
                dnB = bpool.tile([128, T], F32, tag="dnB")
                rcB = bpool.tile([128, T], F32, tag="rcB")
                for half in range(2):
                    mps = rowps.tile([128, 512], F32, tag="mps")
                    nc.tensor.matmul(mps[:], ones_row[0:1, :],
                                     mu_v[:, 512 * half:512 * half + 512, b],
                                     start=True, stop=True)
                    nc.scalar.copy(muB[:, 512 * half:512 * half + 512], mps[:])
                    dps = rowps.tile([128, 512], F32, tag="mps")
                    nc.tensor.matmul(dps[:], ones_row[0:1, :],
                                     den_v[:, 512 * half:512 * half + 512, b],
                                     start=True, stop=True)
                    nc.scalar.copy(dnB[:, 512 * half:512 * half + 512], dps[:])

                wstack = bpool.tile([128, 8 * T], BF16, tag="wstack")
                for tt in range(8):
                    d0 = wk.tile([128, T], F32, tag="d0")
                    nc.vector.tensor_sub(d0[:], relM_sb[:, T * tt:T * tt + T], muB[:])
                    nc.vector.tensor_mul(d0[:], d0[:], d0[:])
                    nc.vector.tensor_mul(d0[:], d0[:], dnB[:])
                    nc.scalar.activation(wstack[:, T * tt:T * tt + T], d0[:],
                                         AF.Exp, scale=-1.0)
                wsmax = nrm.tile([128, T], F32, tag="wsmax")
                for half in range(2):
                    wps = rowps.tile([128, 512], F32, tag="mps")
                    for tt in range(8):
                        nc.tensor.matmul(
                            wps[0:1, :], ones_col[:, 0:1],
                            wstack[:, T * tt + 512 * half:T * tt + 512 * half + 512],
                            start=(tt == 0), stop=(tt == 7))
                    nc.vector.tensor_scalar_max(
                        wsmax[0:1, 512 * half:512 * half + 512], wps[0:1, :],
                        EPS_NORM)
                nc.vector.reciprocal(wsmax[0:1, :], wsmax[0:1, :])
                for half in range(2):
                    rps = rowps.tile([128, 512], F32, tag="mps")
                    nc.tensor.matmul(rps[:], ones_row[0:1, :],
                                     wsmax[0:1, 512 * half:512 * half + 512],
                                     start=True, stop=True)
                    nc.scalar.copy(rcB[:, 512 * half:512 * half + 512], rps[:])

                encnat = bpool.tile([128, 8 * 512], BF16, tag="encnat")
                for tt in range(8):
                    for c in range(4):
                        tps = tps_pool.tile([128, 128], BF16)
                        nc.tensor.transpose(
                            tps[:], encT_v[:, 128 * tt:128 * tt + 128, 4 * c + b],
                            ident[:])
                        nc.scalar.copy(
                            encnat[:, 512 * tt + 128 * c:512 * tt + 128 * c + 128],
                            tps[:])

                for hc in range(4):
                    for half in range(2):
                        cps = ps512.tile([128, 512], F32)
                        for tt in range(8):
                            nc.tensor.matmul(
                                cps[:],
                                encnat[:, 512 * tt + 128 * hc:512 * tt + 128 * hc + 128],
                                wstack[:, T * tt + 512 * half:T * tt + 512 * half + 512],
                                start=(tt == 0), stop=(tt == 7))
                        nc.vector.tensor_mul(
                            ctxTs[b][:, T * hc + 512 * half:T * hc + 512 * half + 512],
                            cps[:], rcB[:, 512 * half:512 * half + 512])

        # ================= Phase 3b: combined + decoder =====================
        with ExitStack() as p4:
            c4 = p4.enter_context(tc.tile_pool(name="p4c", bufs=1))
            wc_sb = c4.tile([128, 8 * 4 * 128], BF16)
            nc.sync.dma_start(wc_sb[:], wcT_in[:, :])
            bc_sb = c4.tile([128, 4], F32)
            nc.sync.dma_start(bc_sb[:], bc_in[:, :])
            emb_sb = c4.tile([128, 4 * VSH], BF16)
            nc.sync.dma_start(emb_sb[:], embT_in[:, :])
            bwork = p4.enter_context(tc.tile_pool(name="p4b", bufs=1))
            dec_e = p4.enter_context(tc.tile_pool(name="p4d", bufs=4))
            qps_pool = p4.enter_context(tc.tile_pool(name="qps", bufs=3, space="PSUM"))

            for b in range(B):
                combT = bwork.tile([128, 4 * T], BF16, tag="combT")
                for m in range(4):
                    for half in range(2):
                        qps = qps_pool.tile([128, 512], F32, tag="q")
                        for k in range(8):
                            if k < 4:
                                rhs = ctxTs[b][:, T * k + 512 * half:
                                               T * k + 512 * half + 512]
                            else:
                                rhs = encT_v[:, 512 * half:512 * half + 512,
                                             4 * (k - 4) + b]
                            nc.tensor.matmul(
                                qps[:],
                                wc_sb[:, (k * 4 + m) * 128:(k * 4 + m + 1) * 128],
                                rhs, start=(k == 0), stop=(k == 7))
                        nc.scalar.activation(
                            combT[:, T * m + 512 * half:T * m + 512 * half + 512],
                            qps[:], AF.Tanh, bias=bc_sb[:, m:m + 1])

                for tc8 in range(8):
                    for vc in range(8):
                        dps = qps_pool.tile([128, 500], F32, tag="q")
                        for k in range(4):
                            nc.tensor.matmul(
                                dps[:],
                                combT[:, T * k + 128 * tc8:T * k + 128 * tc8 + 128],
                                emb_sb[:, VSH * k + 500 * vc:VSH * k + 500 * vc + 500],
                                start=(k == 0), stop=(k == 3))
                        oe = dec_e.tile([128, 500], F32, tag="oe")
                        nc.scalar.copy(oe[:], dps[:])
                        nc.sync.dma_start(
                            logits_out[T * b + 128 * tc8:T * b + 128 * tc8 + 128,
                                       500 * vc:500 * vc + 500],
                            oe[:])

    nc.finalize()
    return nc


_NC_CACHE = [None]


def _get_nc():
    if _NC_CACHE[0] is None:
        _NC_CACHE[0] = build_nc()
    return _NC_CACHE[0]


def kernel(input_ids, pad_lengths, emb, dec_bias, Wih, Whh, bih, bhh,
           Wp_ih, Wp_hh, bp_ih, bp_hh, Wmu, bmu, Wsig, bsig, Wc, bc):
    input_ids = np.asarray(input_ids)
    pad_lengths = np.asarray(pad_lengths)
    emb = _f32(emb); dec_bias = _f32(dec_bias)
    Wih = _f32(Wih); Whh = _f32(Whh); bih = _f32(bih); bhh = _f32(bhh)
    Wp_ih = _f32(Wp_ih); Wp_hh = _f32(Wp_hh); bp_ih = _f32(bp_ih); bp_hh = _f32(bp_hh)
    Wmu = _f32(Wmu); bmu = _f32(bmu); Wsig = _f32(Wsig); bsig = _f32(bsig)
    Wc = _f32(Wc); bc = _f32(bc)

    perm = np.r_[0:H, H:2 * H, 3 * H:4 * H, 2 * H:3 * H]
    permp = np.r_[0:P, P:2 * P, 3 * P:4 * P, 2 * P:3 * P]

    x = emb[input_ids]                                   # [B,T,H]
    xT = x.reshape(B, T, 4, 128).transpose(3, 2, 0, 1).reshape(128, 4 * B * T)

    def pack_kxm(Wt, nk, nm):
        return Wt.reshape(nk, 128, nm, 128).transpose(1, 0, 2, 3).reshape(
            128, nk * nm * 128)

    wihT = pack_kxm(Wih[perm].T, 4, 16)
    whhT = pack_kxm(Whh[perm].T, 4, 16)
    mbias = (bih + bhh)[perm].reshape(16, 128).T

    wpihT = Wp_ih[permp].reshape(4, P, 4, 128).transpose(3, 2, 0, 1).reshape(
        128, 4 * 4 * P)
    wphhT = Wp_hh[permp].T                               # [20, 80]
    w3T = np.vstack([Wmu, Wsig]).T                       # [20, 4]
    bpv = (bp_ih + bp_hh)[permp]
    bp_t = np.zeros((P, 16), np.float32)
    for g in range(4):
        for bb in range(4):
            bp_t[:, 4 * g + bb] = bpv[P * g:P * (g + 1)]
    bm4 = np.concatenate([bmu, bsig])
    bm_t = np.repeat(bm4[:, None], 4, axis=1).reshape(1, 16)

    invL = (1.0 / pad_lengths.astype(np.float64)).astype(np.float32).reshape(1, 4)
    j1 = np.arange(1, T + 1, dtype=np.float32).reshape(1, T)

    ti = np.arange(T, dtype=np.float64)
    relM = (ti[:, None] / (ti[None, :] + 1.0)).astype(np.float32)
    relM[ti[:, None] > ti[None, :]] = 1e9
    relM_p = relM.reshape(8, 128, T).transpose(1, 0, 2).reshape(128, 8 * T)

    wcT = Wc.reshape(4, 128, 8, 128).transpose(3, 2, 0, 1).reshape(128, 8 * 4 * 128)
    bc_t = bc.reshape(4, 128).T

    common = {
        "xT": _bf(xT), "wihT": _bf(wihT), "whhT": _bf(whhT),
        "mbias": _f32(mbias), "wpihT": _bf(wpihT), "wphhT": _bf(wphhT),
        "w3T": _bf(w3T), "bp": _f32(bp_t), "bm": _f32(bm_t),
        "invL": invL, "j1": j1, "relM": _f32(relM_p),
        "wcT": _bf(wcT), "bc": _f32(bc_t),
    }
    in_maps = []
    for c in range(NCORES):
        sh = emb[VSH * c:VSH * (c + 1)]
        embT = sh.reshape(VSH, 4, 128).transpose(2, 1, 0).reshape(128, 4 * VSH)
        m = dict(common)
        m["embT"] = _bf(embT)
        in_maps.append(m)

    nc = _get_nc()
    trace = bool(os.environ.get("KERNEL_TRACE"))
    res = run_bass_kernel_spmd(nc, in_maps, core_ids=list(range(NCORES)),
                               trace=trace)
    LAST_EXEC_NS[0] = res.exec_time_ns

    parts = [res.results[c]["logits"].reshape(B, T, VSH) for c in range(NCORES)]
    logits = np.concatenate(parts, axis=-1).astype(np.float32)
    if np.any(dec_bias):
        logits = logits + dec_bias
    return logits

